# revision 4
# baseline (speedup 1.0000x reference)
"""Cosine attention kernel for Trainium2 (Bass/Tile), SPMD over 8 NeuronCores.

Problem: B=16, Tq=Tk=2048, D=128, fp32.
  q_n = q / ||q||, k_n = k / ||k||  (row-wise L2, eps negligible in fp32)
  scores = q_n @ k_n^T              (B, Tq, Tk), values in [-1, 1]
  attn   = softmax(scores, -1)      (max-subtraction skipped: scores bounded)
  out    = attn @ v
Returns (out, attn) like the reference.

Sharding: data-parallel over batch; each of the 8 cores owns 2 batches.

Per-core design (all fp32, no big transposes, no extra DMA traffic):
  - Normalize q,k in natural layout (sumsq on DVE, 1/sqrt as exp(-0.5*ln) on
    ACT - keeps everything in the natural_log_exp table set, zero switches).
  - Transpose q_n,k_n 128x128 tiles on PE -> qnT,knT [D, T] layouts.
  - Scores are computed TWICE on PE (PE has slack; the alternative - a big
    transpose of the exp'd matrix - is strictly worse on every engine):
      pass 1 [k,q]: exp -> eT feeds attn@v (v-stationary accumulation).
      pass 2 [q,k]: exp with accum_out -> row sums for free; scale by 1/Z on
      DVE (2x fp32 mode) -> attention rows DMA out at full line rate.
  - Output comes out [d, q]; transposed back on PE, scaled by 1/Z from PSUM.
The kernel is DMA-bound by the 32MB/core attention write (~358 GB/s/core HBM);
ACT (two exp passes) is the busiest compute engine, PE/DVE fit underneath.
"""

import sys

sys.path.insert(0, "/opt/trn_rl_repo")

from contextlib import ExitStack

import numpy as np

import concourse.bass as bass
import concourse.mybir as mybir
import concourse.tile as tile
from concourse import bacc
from concourse.bass_utils import run_bass_kernel_spmd
from concourse.masks import make_identity

F32 = mybir.dt.float32
P = 128            # partitions / head dim
T = 2048           # Tq = Tk
NB = 16            # tiles of 128 along T
B_FULL = 16
N_CORES = 8
BPC = B_FULL // N_CORES  # batches per core


def _build_body(ctx: ExitStack, tc: tile.TileContext, q_d, k_d, v_d, out_d, attn_d):
    nc = tc.nc
    EXP = mybir.ActivationFunctionType.Exp
    LN = mybir.ActivationFunctionType.Ln
    ADD = mybir.AluOpType.add
    MULT = mybir.AluOpType.mult

    singles = ctx.enter_context(tc.tile_pool(name="singles", bufs=1))
    nat = ctx.enter_context(tc.tile_pool(name="nat", bufs=2))
    qkT = ctx.enter_context(tc.tile_pool(name="qkT", bufs=2 * BPC))
    vpool = ctx.enter_context(tc.tile_pool(name="vpool", bufs=2))
    eTp = ctx.enter_context(tc.tile_pool(name="eT", bufs=4))
    epool = ctx.enter_context(tc.tile_pool(name="e", bufs=3))
    oTsb = ctx.enter_context(tc.tile_pool(name="oTsb", bufs=2))
    outsb = ctx.enter_context(tc.tile_pool(name="outsb", bufs=2))
    small = ctx.enter_context(tc.tile_pool(name="small", bufs=4 * BPC))
    sqs = ctx.enter_context(tc.tile_pool(name="sqs", bufs=2))
    # PSUM: scores 3x[128,1024] = 6 banks, oT accumulators 2x[128,512] = 2.
    ps_scores = ctx.enter_context(tc.tile_pool(name="scores", bufs=3, space="PSUM"))
    ps_oT = ctx.enter_context(tc.tile_pool(name="psoT", bufs=2, space="PSUM"))

    identity = singles.tile([P, P], F32)
    make_identity(nc, identity[:])

    # ---- prologue: normalized + transposed q/k for all batches ----
    tT = {}
    for b in range(BPC):
        for name, dram in (("q", q_d), ("k", k_d)):
            natt = nat.tile([P, NB, P], F32, tag="nat")
            nc.sync.dma_start(natt[:], dram[b].rearrange("(t p) d -> p t d", p=P))
            ss = small.tile([P, NB], F32, tag="ss")
            sqo = sqs.tile([P, NB, P], F32, tag="sqo")
            nc.vector.tensor_mul(sqo[:], natt[:], natt[:])
            nc.vector.tensor_reduce(ss[:], sqo[:], axis=mybir.AxisListType.X, op=ADD)
            # 1/||x|| = exp(-0.5 * ln(sumsq)); stays in the ln+exp table set
            lns = small.tile([P, NB], F32, tag="lns")
            nc.scalar.activation(lns[:], ss[:], LN)
            rn = small.tile([P, NB], F32, tag="rn")
            nc.scalar.activation(rn[:], lns[:], EXP, scale=-0.5)
            t_tile = qkT.tile([P, T], F32, tag="qkT")
            for i in range(NB):
                nc.vector.tensor_scalar_mul(natt[:, i, :], natt[:, i, :], rn[:, i : i + 1])
                pst = ps_scores.tile([P, 1024], F32, tag="s")
                nc.tensor.transpose(pst[:, 0:P], natt[:, i, :], identity[:])
                nc.vector.tensor_copy(t_tile[:, i * P : (i + 1) * P], pst[:, 0:P])
            tT[(b, name)] = t_tile

    # ---- main loop ----
    for b in range(BPC):
        qnT = tT[(b, "q")]
        knT = tT[(b, "k")]
        vt = vpool.tile([P, NB, P], F32, tag="v")
        nc.sync.dma_start(vt[:], v_d[b].rearrange("(t p) d -> p t d", p=P))
        rcpZ = small.tile([P, NB], F32, tag="rcpZ")
        oT_sb = oTsb.tile([P, T], F32, tag="oTsb")
        out_sb = outsb.tile([P, NB, P], F32, tag="outsb")

        # scores^T [k,q] -> exp -> eT; attn_unnorm @ v accumulated v-stationary
        for qh in range(2):  # q halves of 1024
            q0 = qh * 1024
            oT0 = ps_oT.tile([P, 512], F32, tag="oT")
            oT1 = ps_oT.tile([P, 512], F32, tag="oT")
            for kb in range(NB):
                sT = ps_scores.tile([P, 1024], F32, tag="s")
                kcol = knT[:, kb * P : (kb + 1) * P]
                nc.tensor.matmul(sT[:, 0:512], kcol, qnT[:, q0 : q0 + 512])
                nc.tensor.matmul(sT[:, 512:1024], kcol, qnT[:, q0 + 512 : q0 + 1024])
                eT = eTp.tile([P, 1024], F32, tag="eT")
                nc.scalar.activation(eT[:], sT[:], EXP)
                nc.tensor.matmul(
                    oT0[:], vt[:, kb, :], eT[:, 0:512],
                    start=(kb == 0), stop=(kb == NB - 1),
                )
                nc.tensor.matmul(
                    oT1[:], vt[:, kb, :], eT[:, 512:1024],
                    start=(kb == 0), stop=(kb == NB - 1),
                )
            nc.vector.tensor_copy(oT_sb[:, q0 : q0 + 512], oT0[:])
            nc.vector.tensor_copy(oT_sb[:, q0 + 512 : q0 + 1024], oT1[:])

        # scores [q,k] -> exp(+accum) -> normalized attention rows; output
        for qb in range(NB):
            et = epool.tile([P, T], F32, tag="e")
            zp = small.tile([P, 2], F32, tag="zp")
            qcol = qnT[:, qb * P : (qb + 1) * P]
            for kc in range(2):  # k halves of 1024
                k0 = kc * 1024
                sp = ps_scores.tile([P, 1024], F32, tag="s")
                nc.tensor.matmul(sp[:, 0:512], qcol, knT[:, k0 : k0 + 512])
                nc.tensor.matmul(sp[:, 512:1024], qcol, knT[:, k0 + 512 : k0 + 1024])
                nc.scalar.activation(
                    et[:, k0 : k0 + 1024], sp[:], EXP, accum_out=zp[:, kc : kc + 1]
                )
            z = small.tile([P, 1], F32, tag="z")
            nc.vector.tensor_add(z[:], zp[:, 0:1], zp[:, 1:2])
            nc.vector.reciprocal(rcpZ[:, qb : qb + 1], z[:])
            nc.vector.tensor_scalar_mul(et[:], et[:], rcpZ[:, qb : qb + 1])
            nc.sync.dma_start(attn_d[b, qb * P : (qb + 1) * P, :], et[:])
            # output block: transpose [d,q]->[q,d] on PE, scale by 1/Z
            op = ps_scores.tile([P, 1024], F32, tag="s")
            nc.tensor.transpose(op[:, 0:P], oT_sb[:, qb * P : (qb + 1) * P], identity[:])
            nc.vector.tensor_scalar_mul(out_sb[:, qb, :], op[:, 0:P], rcpZ[:, qb : qb + 1])
        nc.sync.dma_start(out_d[b].rearrange("(t p) d -> p t d", p=P), out_sb[:])


_BUILT = {}


def _get_nc():
    if "nc" not in _BUILT:
        nc = bacc.Bacc("TRN2", target_bir_lowering=False, debug=False)
        q_d = nc.declare_dram_parameter("q", [BPC, T, P], F32, isOutput=False)
        k_d = nc.declare_dram_parameter("k", [BPC, T, P], F32, isOutput=False)
        v_d = nc.declare_dram_parameter("v", [BPC, T, P], F32, isOutput=False)
        out_d = nc.declare_dram_parameter("out", [BPC, T, P], F32, isOutput=True)
        attn_d = nc.declare_dram_parameter("attn", [BPC, T, T], F32, isOutput=True)
        with tile.TileContext(nc) as tc:
            with ExitStack() as ctx:
                _build_body(ctx, tc, q_d.ap(), k_d.ap(), v_d.ap(), out_d.ap(), attn_d.ap())
        if not nc.is_finalized():
            nc.finalize()
        _BUILT["nc"] = nc
    return _BUILT["nc"]


def kernel(q, k, v, _trace=False):
    q = np.ascontiguousarray(np.asarray(q, dtype=np.float32))
    k = np.ascontiguousarray(np.asarray(k, dtype=np.float32))
    v = np.ascontiguousarray(np.asarray(v, dtype=np.float32))
    nc = _get_nc()
    in_maps = [
        {
            "q": q[c * BPC : (c + 1) * BPC],
            "k": k[c * BPC : (c + 1) * BPC],
            "v": v[c * BPC : (c + 1) * BPC],
        }
        for c in range(N_CORES)
    ]
    res = run_bass_kernel_spmd(nc, in_maps, list(range(N_CORES)), trace=_trace)
    out = np.concatenate([r["out"] for r in res.results], axis=0)
    attn = np.concatenate([r["attn"] for r in res.results], axis=0)
    if _trace:
        kernel.last_exec_time_ns = res.exec_time_ns
        kernel.last_results = res
    return out, attn


# revision 9
# speedup vs baseline: 1.7050x; 1.7050x over previous
"""Cosine attention kernel for Trainium2 (Bass/Tile), SPMD over 8 NeuronCores.

Problem: B=16, Tq=Tk=2048, D=128, fp32.
  q_n = q / ||q||, k_n = k / ||k||  (row-wise L2, eps negligible in fp32)
  scores = q_n @ k_n^T              (B, Tq, Tk), values in [-1, 1]
  attn   = softmax(scores, -1)      (max-subtraction skipped: scores bounded)
  out    = attn @ v
Returns (out, attn) like the reference.

Sharding: data-parallel over batch; each of the 8 cores owns 2 batches.

Per-core design (all fp32, no big transposes, no extra DMA traffic):
  - Normalize q,k in natural layout (sumsq on DVE, 1/sqrt as exp(-0.5*ln) on
    ACT - keeps everything in the natural_log_exp table set, zero switches).
  - Transpose q_n,k_n 128x128 tiles on PE -> qnT,knT [D, T] layouts.
  - Scores are computed TWICE on PE (PE has slack; the alternative - a big
    transpose of the exp'd matrix - is strictly worse on every engine):
      pass 1 [k,q]: exp -> eT feeds attn@v (v-stationary accumulation).
      pass 2 [q,k]: exp with accum_out -> row sums for free; scale by 1/Z on
      DVE (2x fp32 mode) -> attention rows DMA out at full line rate.
  - Output comes out [d, q]; transposed back on PE, scaled by 1/Z from PSUM.
The kernel is DMA-bound by the 32MB/core attention write (~358 GB/s/core HBM);
ACT (two exp passes) is the busiest compute engine, PE/DVE fit underneath.
"""

import sys

sys.path.insert(0, "/opt/trn_rl_repo")

from contextlib import ExitStack

import numpy as np

import concourse.bass as bass
import concourse.mybir as mybir
import concourse.tile as tile
from concourse import bacc
from concourse.bass_utils import run_bass_kernel_spmd
from concourse.masks import make_identity

F32 = mybir.dt.float32
F32R = mybir.dt.float32r
P = 128            # partitions / head dim
T = 2048           # Tq = Tk
NB = 16            # tiles of 128 along T
B_FULL = 16
N_CORES = 8
BPC = B_FULL // N_CORES  # batches per core


def _build_body(ctx: ExitStack, tc: tile.TileContext, q_d, k_d, v_d, out_d, attn_d):
    nc = tc.nc
    EXP = mybir.ActivationFunctionType.Exp
    LN = mybir.ActivationFunctionType.Ln
    ADD = mybir.AluOpType.add
    MULT = mybir.AluOpType.mult

    singles = ctx.enter_context(tc.tile_pool(name="singles", bufs=1))
    nat = ctx.enter_context(tc.tile_pool(name="nat", bufs=2))
    qkT = ctx.enter_context(tc.tile_pool(name="qkT", bufs=2 * BPC))
    vpool = ctx.enter_context(tc.tile_pool(name="vpool", bufs=2))
    eTp = ctx.enter_context(tc.tile_pool(name="eT", bufs=4))
    epool = ctx.enter_context(tc.tile_pool(name="e", bufs=3))
    oTsb = ctx.enter_context(tc.tile_pool(name="oTsb", bufs=2))
    outsb = ctx.enter_context(tc.tile_pool(name="outsb", bufs=2))
    small = ctx.enter_context(tc.tile_pool(name="small", bufs=4 * BPC))
    sqs = ctx.enter_context(tc.tile_pool(name="sqs", bufs=2))
    # PSUM: scores 3x[128,1024] = 6 banks, oT accumulators 2x[128,512] = 2.
    ps_scores = ctx.enter_context(tc.tile_pool(name="scores", bufs=3, space="PSUM"))
    ps_oT = ctx.enter_context(tc.tile_pool(name="psoT", bufs=2, space="PSUM"))

    identity = singles.tile([P, P], F32)
    make_identity(nc, identity[:])

    # ---- prologue: normalized + transposed q/k for all batches ----
    tT = {}
    for b in range(BPC):
        for name, dram in (("q", q_d), ("k", k_d)):
            natt = nat.tile([P, NB, P], F32, tag="nat")
            nc.sync.dma_start(natt[:], dram[b].rearrange("(t p) d -> p t d", p=P))
            ss = small.tile([P, NB], F32, tag="ss")
            sqo = sqs.tile([P, NB, P], F32, tag="sqo")
            nc.vector.tensor_mul(sqo[:], natt[:], natt[:])
            nc.vector.tensor_reduce(ss[:], sqo[:], axis=mybir.AxisListType.X, op=ADD)
            # 1/||x|| = exp(-0.5 * ln(sumsq)); stays in the ln+exp table set
            lns = small.tile([P, NB], F32, tag="lns")
            nc.scalar.activation(lns[:], ss[:], LN)
            rn = small.tile([P, NB], F32, tag="rn")
            nc.scalar.activation(rn[:], lns[:], EXP, scale=-0.5)
            t_tile = qkT.tile([P, T], F32, tag="qkT")
            for i in range(NB):
                nc.vector.tensor_scalar_mul(natt[:, i, :], natt[:, i, :], rn[:, i : i + 1])
                pst = ps_scores.tile([P, 1024], F32, tag="s")
                nc.tensor.transpose(pst[:, 0:P], natt[:, i, :], identity[:])
                nc.vector.tensor_copy(t_tile[:, i * P : (i + 1) * P], pst[:, 0:P])
            t_r = qkT.tile([P, T], F32R, tag="qkTr")
            nc.vector.tensor_copy(t_r[:], t_tile[:])
            tT[(b, name)] = t_r

    # ---- main loop ----
    for b in range(BPC):
        qnT = tT[(b, "q")]
        knT = tT[(b, "k")]
        vt = vpool.tile([P, NB, P], F32, tag="v")
        nc.sync.dma_start(vt[:], v_d[b].rearrange("(t p) d -> p t d", p=P))
        vtr = vpool.tile([P, NB, P], F32R, tag="vr")
        nc.vector.tensor_copy(vtr[:], vt[:])
        rcpZ = small.tile([P, NB], F32, tag="rcpZ")
        oT_sb = oTsb.tile([P, T], F32, tag="oTsb")
        out_sb = outsb.tile([P, NB, P], F32, tag="outsb")

        # scores^T [k,q] -> exp -> eT; attn_unnorm @ v accumulated v-stationary
        for qh in range(2):  # q halves of 1024
            q0 = qh * 1024
            oT0 = ps_oT.tile([P, 512], F32, tag="oT")
            oT1 = ps_oT.tile([P, 512], F32, tag="oT")
            for kb in range(NB):
                sT = ps_scores.tile([P, 1024], F32, tag="s")
                kcol = knT[:, kb * P : (kb + 1) * P]
                nc.tensor.matmul(sT[:, 0:512], kcol, qnT[:, q0 : q0 + 512])
                nc.tensor.matmul(sT[:, 512:1024], kcol, qnT[:, q0 + 512 : q0 + 1024])
                eT = eTp.tile([P, 1024], F32R, tag="eT")
                nc.scalar.activation(eT[:], sT[:], EXP)
                nc.tensor.matmul(
                    oT0[:], vtr[:, kb, :], eT[:, 0:512],
                    start=(kb == 0), stop=(kb == NB - 1),
                )
                nc.tensor.matmul(
                    oT1[:], vtr[:, kb, :], eT[:, 512:1024],
                    start=(kb == 0), stop=(kb == NB - 1),
                )
            nc.vector.tensor_copy(oT_sb[:, q0 : q0 + 512], oT0[:])
            nc.vector.tensor_copy(oT_sb[:, q0 + 512 : q0 + 1024], oT1[:])

        # scores [q,k] -> exp(+accum) -> normalized attention rows; output
        for qb in range(NB):
            et = epool.tile([P, T], F32, tag="e")
            zp = small.tile([P, 2], F32, tag="zp")
            qcol = qnT[:, qb * P : (qb + 1) * P]
            for kc in range(2):  # k halves of 1024
                k0 = kc * 1024
                sp = ps_scores.tile([P, 1024], F32, tag="s")
                nc.tensor.matmul(sp[:, 0:512], qcol, knT[:, k0 : k0 + 512])
                nc.tensor.matmul(sp[:, 512:1024], qcol, knT[:, k0 + 512 : k0 + 1024])
                nc.scalar.activation(
                    et[:, k0 : k0 + 1024], sp[:], EXP, accum_out=zp[:, kc : kc + 1]
                )
            z = small.tile([P, 1], F32, tag="z")
            nc.vector.tensor_add(z[:], zp[:, 0:1], zp[:, 1:2])
            nc.vector.reciprocal(rcpZ[:, qb : qb + 1], z[:])
            nc.vector.tensor_scalar_mul(et[:], et[:], rcpZ[:, qb : qb + 1])
            nc.sync.dma_start(attn_d[b, qb * P : (qb + 1) * P, :], et[:])
            # output block: transpose [d,q]->[q,d] on PE, scale by 1/Z
            op = ps_scores.tile([P, 1024], F32, tag="s")
            nc.tensor.transpose(op[:, 0:P], oT_sb[:, qb * P : (qb + 1) * P], identity[:])
            nc.vector.tensor_scalar_mul(out_sb[:, qb, :], op[:, 0:P], rcpZ[:, qb : qb + 1])
        nc.sync.dma_start(out_d[b].rearrange("(t p) d -> p t d", p=P), out_sb[:])


_BUILT = {}


def _get_nc():
    if "nc" not in _BUILT:
        nc = bacc.Bacc("TRN2", target_bir_lowering=False, debug=False)
        q_d = nc.declare_dram_parameter("q", [BPC, T, P], F32, isOutput=False)
        k_d = nc.declare_dram_parameter("k", [BPC, T, P], F32, isOutput=False)
        v_d = nc.declare_dram_parameter("v", [BPC, T, P], F32, isOutput=False)
        out_d = nc.declare_dram_parameter("out", [BPC, T, P], F32, isOutput=True)
        attn_d = nc.declare_dram_parameter("attn", [BPC, T, T], F32, isOutput=True)
        with tile.TileContext(nc) as tc:
            with ExitStack() as ctx:
                _build_body(ctx, tc, q_d.ap(), k_d.ap(), v_d.ap(), out_d.ap(), attn_d.ap())
        if not nc.is_finalized():
            nc.finalize()
        _BUILT["nc"] = nc
    return _BUILT["nc"]


def kernel(q, k, v, _trace=False):
    q = np.ascontiguousarray(np.asarray(q, dtype=np.float32))
    k = np.ascontiguousarray(np.asarray(k, dtype=np.float32))
    v = np.ascontiguousarray(np.asarray(v, dtype=np.float32))
    nc = _get_nc()
    in_maps = [
        {
            "q": q[c * BPC : (c + 1) * BPC],
            "k": k[c * BPC : (c + 1) * BPC],
            "v": v[c * BPC : (c + 1) * BPC],
        }
        for c in range(N_CORES)
    ]
    res = run_bass_kernel_spmd(nc, in_maps, list(range(N_CORES)), trace=_trace)
    out = np.concatenate([r["out"] for r in res.results], axis=0)
    attn = np.concatenate([r["attn"] for r in res.results], axis=0)
    if _trace:
        kernel.last_exec_time_ns = res.exec_time_ns
        kernel.last_results = res
    return out, attn
